# revision 11
# baseline (speedup 1.0000x reference)
"""CrossScaleBlock (Swin-3D style window-attention transformer block) on 8 trn2 cores.

Sharding: data-parallel over the 256 windows -> 32 windows/core. Params replicated.

Per-core kernel (T = 32*196 = 6272 tokens, C = 384, H = 12 heads, hd = 32):
  P1: LN1 token-major (bn_stats) -> h1 -> PE-transpose -> h1_T [c, T] bf16
  P2: per block of 8 windows: q/k feature-major, v token-major (98-row window chunks)
      per window: scores_T[k,q] (4-head row-packed, K=32) -> exp (ACT) -> *exp(bias) (DVE)
      attn@v token-major with ones-column denominators -> reciprocal -> normalize (DVE)
      -> PE-transpose -> proj token-major -> +x -> x1 -> DRAM scratch
  P3: LN2 -> h2_T -> fc1 feature-major -> GELU -> fc2 token-major -> +x1 -> out
"""

import os
import sys
import numpy as np

sys.path.insert(0, "/opt/trn_rl_repo")

import ml_dtypes

import concourse.bass as bass
import concourse.bacc as bacc
import concourse.mybir as mybir
import concourse.tile as tile
from concourse.bass_utils import run_bass_kernel_spmd

F32 = mybir.dt.float32
BF16 = mybir.dt.bfloat16
AF = mybir.ActivationFunctionType
ALU = mybir.AluOpType
BF16_NP = ml_dtypes.bfloat16

N_CORES = 8
DIM = 384
HEADS = 12
HD = 32
N = 196          # tokens per window
BW = 256         # total windows
WPC = BW // N_CORES   # 32 windows per core
T = WPC * N      # 6272 tokens per core
NT = T // 128    # 49 token tiles of 128
CCH = DIM // 128  # 3 feature chunks
HIDDEN = 4 * DIM
HCH = HIDDEN // 128  # 12
EPS = 1e-5
HK = 98          # half-window token chunk (2 x 98 = 196)
WB = 8           # windows per attention block
NBLK = WPC // WB  # 4
TB = WB * N      # 1568 tokens per block
NTILE_QK = 4     # q/k matmul N tiles per block
NQK = TB // NTILE_QK  # 392


def build_nc():
    nc = bacc.Bacc(trn_type="TRN2", target_bir_lowering=False)

    # ---------------- DRAM I/O ----------------
    x_in = nc.dram_tensor("x_in", [T, DIM], F32, kind="ExternalInput")
    wq = nc.dram_tensor("wq", [128, CCH, DIM], BF16, kind="ExternalInput")
    wk = nc.dram_tensor("wk", [128, CCH, DIM], BF16, kind="ExternalInput")
    wv = nc.dram_tensor("wv", [128, CCH, DIM], BF16, kind="ExternalInput")
    wproj = nc.dram_tensor("wproj", [128, CCH, DIM], BF16, kind="ExternalInput")
    wfc1 = nc.dram_tensor("wfc1", [128, CCH, HIDDEN], BF16, kind="ExternalInput")
    wfc2 = nc.dram_tensor("wfc2", [128, HCH, DIM], BF16, kind="ExternalInput")
    qb_in = nc.dram_tensor("qb", [128, CCH], F32, kind="ExternalInput")
    kb_in = nc.dram_tensor("kb", [128, CCH], F32, kind="ExternalInput")
    fc1b_in = nc.dram_tensor("fc1b", [128, HCH], F32, kind="ExternalInput")
    projb_in = nc.dram_tensor("projb_row", [1, DIM], BF16, kind="ExternalInput")
    fc2b_in = nc.dram_tensor("fc2b_row", [1, DIM], BF16, kind="ExternalInput")
    ebias_in = nc.dram_tensor("ebias", [HK, 2, HEADS, N], BF16, kind="ExternalInput")
    ident_in = nc.dram_tensor("ident", [128, 128], BF16, kind="ExternalInput")
    ones_in = nc.dram_tensor("ones_row", [1, 128], BF16, kind="ExternalInput")
    out_d = nc.dram_tensor("out", [T, DIM], F32, kind="ExternalOutput")

    x_rows = x_in.ap().rearrange("(n p) d -> n p d", p=128)       # [49,128,384]
    x_win = x_in.ap().rearrange("(w c p) d -> w p c d", p=HK, c=2)     # [32,98,2,384]
    out_rows = out_d.ap().rearrange("(n p) d -> n p d", p=128)

    with tile.TileContext(nc) as tc:
        with (
            tc.tile_pool(name="persist", bufs=1) as pp,
            tc.tile_pool(name="dram", bufs=1, space="DRAM") as dp,
        ):
            x1_d = dp.tile([T, DIM], F32, tag="x1_scratch")
            x1_win = x1_d[:].rearrange("(w c p) d -> w p c d", p=HK, c=2)
            x1_rows = x1_d[:].rearrange("(n p) d -> n p d", p=128)

            # persistent SBUF tensors
            s_wq = pp.tile([128, CCH, DIM], BF16, tag="s_wq")
            s_wk = pp.tile([128, CCH, DIM], BF16, tag="s_wk")
            s_wv = pp.tile([128, CCH, DIM], BF16, tag="s_wv")
            s_wproj = pp.tile([128, CCH, DIM], BF16, tag="s_wproj")
            s_wfc1 = pp.tile([128, CCH, HIDDEN], BF16, tag="s_wfc1")
            s_wfc2 = pp.tile([128, HCH, DIM], BF16, tag="s_wfc2")
            s_qb = pp.tile([128, CCH], F32, tag="s_qb")
            s_kb = pp.tile([128, CCH], F32, tag="s_kb")
            s_fc1b = pp.tile([128, HCH], F32, tag="s_fc1b")
            s_projb = pp.tile([1, DIM], BF16, tag="s_projb")
            s_fc2b = pp.tile([1, DIM], BF16, tag="s_fc2b")
            s_eb = pp.tile([HK, 2, HEADS, N], BF16, tag="s_eb")
            s_id = pp.tile([128, 128], BF16, tag="s_id")
            s_ones = pp.tile([1, 128], BF16, tag="s_ones")
            s_eps = pp.tile([128, 1], F32, tag="s_eps")
            s_idf = pp.tile([128, 128], F32, tag="s_idf")
            nc.vector.memset(s_eps[:], EPS)
            h1_T = pp.tile([128, CCH, T], BF16, tag="h1_T")

            for dst, src in [
                (s_wq, wq), (s_wk, wk), (s_wv, wv), (s_wproj, wproj),
                (s_wfc1, wfc1), (s_wfc2, wfc2), (s_qb, qb_in), (s_kb, kb_in),
                (s_fc1b, fc1b_in), (s_projb, projb_in), (s_fc2b, fc2b_in),
                (s_eb, ebias_in), (s_id, ident_in), (s_ones, ones_in),
            ]:
                nc.sync.dma_start(dst[:], src.ap())
            nc.any.tensor_copy(out=s_idf[:], in_=s_id[:])

            # ---------------- P1: LN1 + transpose to h1_T ----------------
            with (
                tc.tile_pool(name="p1", bufs=3) as p1,
                tc.tile_pool(name="p1b", bufs=2) as p1b,
                tc.tile_pool(name="p1ps", bufs=2, space="PSUM") as p1ps,
            ):
                for t in range(NT):
                    xt = p1.tile([128, DIM], F32, tag="xt")
                    nc.sync.dma_start(xt[:], x_rows[t])
                    st6 = p1.tile([128, 6], F32, tag="st6")
                    st2 = p1.tile([128, 4], F32, tag="st2")
                    nc.vector.bn_stats(st6[:], xt[:])
                    nc.vector.bn_aggr(st2[:, :2], st6[:])
                    # rstd = exp(-0.5 * ln(var + eps)) (avoids sqrt table set)
                    nc.scalar.activation(st2[:, 2:3], st2[:, 1:2], AF.Ln, bias=s_eps[:])
                    nc.scalar.activation(st2[:, 3:4], st2[:, 2:3], AF.Exp, scale=-0.5)
                    ht = p1b.tile([128, DIM], F32, tag="ht")
                    hc = p1b.tile([128, DIM], F32, tag="hc")
                    nc.vector.tensor_scalar_sub(hc[:], xt[:], st2[:, 0:1])
                    nc.vector.tensor_scalar_mul(ht[:], hc[:], st2[:, 3:4])
                    for j in range(CCH):
                        tp = p1ps.tile([128, 128], F32, tag="tp")
                        nc.tensor.transpose(tp[:], ht[:, 128 * j:128 * (j + 1)], s_idf[:])
                        nc.any.tensor_copy(
                            out=h1_T[:, j, 128 * t:128 * (t + 1)], in_=tp[:]
                        )

            # ---------------- P2: attention ----------------
            with (
                tc.tile_pool(name="qk", bufs=2) as qkp,
                tc.tile_pool(name="vb", bufs=2) as vbp,
                tc.tile_pool(name="pb", bufs=2) as pbp,
                tc.tile_pool(name="sm", bufs=3) as smp,
                tc.tile_pool(name="aw", bufs=2) as awp,
                tc.tile_pool(name="scps", bufs=1, space="PSUM") as scps,
                tc.tile_pool(name="qtps", bufs=2, space="PSUM") as qtps,
                tc.tile_pool(name="avps", bufs=1, space="PSUM") as avps,
                tc.tile_pool(name="prps", bufs=1, space="PSUM") as prps,
            ):
                for b in range(NBLK):
                    tok0 = b * TB
                    q_fm = qkp.tile([128, CCH, TB], BF16, tag="q_fm")
                    k_fm = qkp.tile([128, CCH, TB], BF16, tag="k_fm")
                    # q/k feature-major: out[feat_chunk, tokens]
                    for dst, w_s, b_s in ((q_fm, s_wq, s_qb), (k_fm, s_wk, s_kb)):
                        for m in range(CCH):
                            for nt_i in range(NTILE_QK):
                                ps = qtps.tile([128, 512], F32, tag="qt")
                                for kc in range(CCH):
                                    nc.tensor.matmul(
                                        ps[:, :NQK],
                                        lhsT=w_s[:, kc, 128 * m:128 * (m + 1)],
                                        rhs=h1_T[:, kc, tok0 + NQK * nt_i:tok0 + NQK * (nt_i + 1)],
                                        start=(kc == 0), stop=(kc == CCH - 1),
                                    )
                                nc.scalar.activation(
                                    dst[:, m, NQK * nt_i:NQK * (nt_i + 1)],
                                    ps[:, :NQK], AF.Identity, bias=b_s[:, m:m + 1],
                                )
                    # v token-major with ones column: v_buf [98, WB, 2, 12, 33]
                    v_buf = vbp.tile([HK, WB, 2, HEADS, HD + 1], BF16, tag="v_buf")
                    nc.vector.memset(v_buf[:, :, :, :, HD:HD + 1], 1.0)
                    for wi in range(WB):
                        for kc in range(2):
                            ps = qtps.tile([128, 512], F32, tag="qt")
                            pv = ps[:HK, :DIM]
                            for cc in range(CCH):
                                nc.tensor.matmul(
                                    pv,
                                    lhsT=h1_T[:, cc, tok0 + N * wi + HK * kc:
                                              tok0 + N * wi + HK * (kc + 1)],
                                    rhs=s_wv[:, cc, :],
                                    start=(cc == 0), stop=(cc == CCH - 1),
                                )
                            nc.any.tensor_copy(
                                out=v_buf[:, wi, kc, :, :HD],
                                in_=pv.rearrange("p (h d) -> p h d", h=HEADS),
                            )

                    for wi in range(WB):
                        w0 = N * wi  # block-local token offset of window
                        p_buf = pbp.tile([HK, 2, HEADS, N], BF16, tag="p_buf")
                        # --- scores_T + exp + bias-mult, per (kc, head-group) ---
                        for kc in range(2):
                            for g in range(HEADS // 4):
                                sc = scps.tile([128, 4, 512], F32, tag="sc")
                                for m in range(4):
                                    h = 4 * g + m
                                    j, r = h // 4, h % 4
                                    nc.tensor.matmul(
                                        sc[:HK, m, :N],
                                        lhsT=k_fm[32 * r:32 * (r + 1), j,
                                                  w0 + HK * kc:w0 + HK * (kc + 1)],
                                        rhs=q_fm[32 * r:32 * (r + 1), j, w0:w0 + N],
                                        start=True, stop=True,
                                        tile_position=(32 * r, 0),
                                    )
                                es = smp.tile([HK, 4, N], BF16, tag="es")
                                nc.scalar.activation(es[:], sc[:HK, :, :N], AF.Exp)
                                nc.vector.tensor_tensor(
                                    p_buf[:, kc, 4 * g:4 * (g + 1), :],
                                    es[:],
                                    s_eb[:, kc, 4 * g:4 * (g + 1), :],
                                    ALU.mult,
                                )
                        # --- attn @ v_aug, token-major out [q, 12, 33] ---
                        at_T = awp.tile([128, CCH, N], BF16, tag="at_T")
                        for qc in range(2):
                            av = avps.tile([HK, HEADS, HD + 1], F32, tag="av")
                            for kc in range(2):
                                for h in range(HEADS):
                                    nc.tensor.matmul(
                                        av[:, h, :],
                                        lhsT=p_buf[:, kc, h, HK * qc:HK * (qc + 1)],
                                        rhs=v_buf[:, wi, kc, h, :],
                                        start=(kc == 0), stop=(kc == 1),
                                        skip_group_check=True,
                                    )
                            rre = smp.tile([HK, HEADS], F32, tag="rre")
                            nc.vector.reciprocal(rre[:], av[:, :, HD])
                            atm = awp.tile([HK, HEADS, HD], F32, tag="atm")
                            nc.vector.tensor_tensor(
                                atm[:], av[:, :, :HD],
                                rre[:, :, None].to_broadcast((HK, HEADS, HD)),
                                ALU.mult,
                            )
                            # transpose to feature-major attn_T [c, 3, q]
                            flat = atm[:].rearrange("p h d -> p (h d)")
                            for j in range(CCH):
                                tp = qtps.tile([128, 512], F32, tag="qt")
                                nc.tensor.transpose(
                                    tp[:, :HK], flat[:, 128 * j:128 * (j + 1)],
                                    s_idf[:HK, :HK],
                                )
                                nc.any.tensor_copy(
                                    out=at_T[:, j, HK * qc:HK * (qc + 1)],
                                    in_=tp[:, :HK],
                                )
                        # --- proj token-major + bias row + residual ---
                        xw = smp.tile([HK, 2, DIM], F32, tag="xw")
                        nc.sync.dma_start(xw[:], x_win[b * WB + wi])
                        x1w = smp.tile([HK, 2, DIM], F32, tag="x1w")
                        for qc in range(2):
                            pr = prps.tile([HK, DIM], F32, tag="pr")
                            for cc in range(CCH):
                                nc.tensor.matmul(
                                    pr[:],
                                    lhsT=at_T[:, cc, HK * qc:HK * (qc + 1)],
                                    rhs=s_wproj[:, cc, :],
                                    start=(cc == 0), stop=False,
                                )
                            nc.tensor.matmul(
                                pr[:],
                                lhsT=s_ones[:, :HK],
                                rhs=s_projb[:],
                                start=False, stop=True,
                            )
                            nc.vector.tensor_tensor(
                                x1w[:, qc, :], pr[:], xw[:, qc, :], ALU.add
                            )
                        nc.sync.dma_start(x1_win[b * WB + wi], x1w[:])

            # ---------------- P3: MLP ----------------
            with (
                tc.tile_pool(name="m1", bufs=2) as m1,
                tc.tile_pool(name="m2", bufs=2) as m2,
                tc.tile_pool(name="m3", bufs=3) as m3,
                tc.tile_pool(name="f1ps", bufs=2, space="PSUM") as f1ps,
                tc.tile_pool(name="f2ps", bufs=2, space="PSUM") as f2ps,
                tc.tile_pool(name="tp3", bufs=2, space="PSUM") as tp3,
            ):
                groups = [(g * 4, 4) for g in range(NT // 4)]
                if NT % 4:
                    groups.append((NT - NT % 4, NT % 4))
                for t0, gn in groups:
                    gtok = 128 * gn
                    x1g = m1.tile([128, 4, DIM], F32, tag="x1g")
                    for i in range(gn):
                        nc.sync.dma_start(x1g[:, i, :], x1_rows[t0 + i])
                    h2_T = m2.tile([128, CCH, 4 * 128], BF16, tag="h2_T")
                    for i in range(gn):
                        st6 = m3.tile([128, 6], F32, tag="st6m")
                        st2 = m3.tile([128, 4], F32, tag="st2m")
                        nc.vector.bn_stats(st6[:], x1g[:, i, :])
                        nc.vector.bn_aggr(st2[:, :2], st6[:])
                        nc.scalar.activation(st2[:, 2:3], st2[:, 1:2], AF.Ln, bias=s_eps[:])
                        nc.scalar.activation(st2[:, 3:4], st2[:, 2:3], AF.Exp, scale=-0.5)
                        h2t = m3.tile([128, DIM], F32, tag="h2t")
                        h2c = m3.tile([128, DIM], F32, tag="h2c")
                        nc.vector.tensor_scalar_sub(h2c[:], x1g[:, i, :], st2[:, 0:1])
                        nc.vector.tensor_scalar_mul(h2t[:], h2c[:], st2[:, 3:4])
                        for j in range(CCH):
                            tp = tp3.tile([128, 128], F32, tag="tpm")
                            nc.tensor.transpose(tp[:], h2t[:, 128 * j:128 * (j + 1)], s_idf[:])
                            nc.any.tensor_copy(
                                out=h2_T[:, j, 128 * i:128 * (i + 1)], in_=tp[:]
                            )
                    g_T = m2.tile([128, HCH, 4 * 128], BF16, tag="g_T")
                    for m in range(HCH):
                        f1 = f1ps.tile([128, 512], F32, tag="f1")
                        for kc in range(CCH):
                            nc.tensor.matmul(
                                f1[:, :gtok],
                                lhsT=s_wfc1[:, kc, 128 * m:128 * (m + 1)],
                                rhs=h2_T[:, kc, :gtok],
                                start=(kc == 0), stop=(kc == CCH - 1),
                            )
                        nc.scalar.activation(
                            g_T[:, m, :gtok], f1[:, :gtok], AF.Gelu,
                            bias=s_fc1b[:, m:m + 1],
                        )
                    for i in range(gn):
                        f2 = f2ps.tile([128, DIM], F32, tag="f2")
                        for kc in range(HCH):
                            nc.tensor.matmul(
                                f2[:],
                                lhsT=g_T[:, kc, 128 * i:128 * (i + 1)],
                                rhs=s_wfc2[:, kc, :],
                                start=(kc == 0), stop=False,
                                skip_group_check=True,
                            )
                        nc.tensor.matmul(
                            f2[:], lhsT=s_ones[:], rhs=s_fc2b[:],
                            start=False, stop=True,
                            skip_group_check=True,
                        )
                        ot = m3.tile([128, DIM], F32, tag="ot")
                        nc.vector.tensor_tensor(ot[:], f2[:], x1g[:, i, :], ALU.add)
                        nc.sync.dma_start(out_rows[t0 + i], ot[:])
    nc.compile()
    return nc


def _prep_host(inputs):
    """Build the per-core in_maps (host-side shard + weight prep)."""
    x = np.ascontiguousarray(np.asarray(inputs["x"], np.float32))  # [256,196,384]
    n1w = np.asarray(inputs["norm1_w"], np.float32)
    n1b = np.asarray(inputs["norm1_b"], np.float32)
    qkv_w = np.asarray(inputs["qkv_w"], np.float32)
    qkv_b = np.asarray(inputs["qkv_b"], np.float32)
    proj_w = np.asarray(inputs["proj_w"], np.float32)
    proj_b = np.asarray(inputs["proj_b"], np.float32)
    rpb = np.asarray(inputs["rpb_table"], np.float32)
    n2w = np.asarray(inputs["norm2_w"], np.float32)
    n2b = np.asarray(inputs["norm2_b"], np.float32)
    fc1_w = np.asarray(inputs["fc1_w"], np.float32)
    fc1_b = np.asarray(inputs["fc1_b"], np.float32)
    fc2_w = np.asarray(inputs["fc2_w"], np.float32)
    fc2_b = np.asarray(inputs["fc2_b"], np.float32)
    rpi = np.asarray(inputs["rpi"]).astype(np.int64)

    scale = HD ** -0.5
    qkv_w_eff = n1w[:, None] * qkv_w
    qkv_b_eff = (qkv_b + n1b @ qkv_w).copy()
    qkv_w_eff[:, :DIM] *= scale
    qkv_b_eff[:DIM] *= scale
    fc1_w_eff = n2w[:, None] * fc1_w
    fc1_b_eff = fc1_b + n2b @ fc1_w
    projb_eff = proj_b + qkv_b_eff[2 * DIM:] @ proj_w  # fold v-bias through proj

    def kxm(w, chunks):  # [K, M] -> [128, chunks, M]
        return np.ascontiguousarray(
            w.reshape(chunks, 128, -1).transpose(1, 0, 2)).astype(BF16_NP)

    def col(bvec):  # [M] -> [128, chunks] fp32
        return np.ascontiguousarray(bvec.reshape(-1, 128).T).astype(np.float32)

    bias = rpb[rpi]                           # [196,196,12]  (q,k,h)
    e_T = np.exp(bias).transpose(2, 1, 0)     # [12,k,q]
    e_T = np.ascontiguousarray(
        e_T.reshape(HEADS, 2, HK, N).transpose(2, 1, 0, 3)).astype(BF16_NP)

    shared = {
        "wq": kxm(qkv_w_eff[:, :DIM], CCH),
        "wk": kxm(qkv_w_eff[:, DIM:2 * DIM], CCH),
        "wv": kxm(qkv_w_eff[:, 2 * DIM:], CCH),
        "wproj": kxm(proj_w, CCH),
        "wfc1": kxm(fc1_w_eff, CCH),
        "wfc2": kxm(fc2_w, HCH),
        "qb": col(qkv_b_eff[:DIM]),
        "kb": col(qkv_b_eff[DIM:2 * DIM]),
        "fc1b": col(fc1_b_eff),
        "projb_row": np.ascontiguousarray(projb_eff[None, :]).astype(BF16_NP),
        "fc2b_row": np.ascontiguousarray(fc2_b[None, :]).astype(BF16_NP),
        "ebias": e_T,
        "ident": np.eye(128, dtype=BF16_NP),
        "ones_row": np.ones((1, 128), BF16_NP),
    }
    in_maps = []
    for c in range(N_CORES):
        m = dict(shared)
        m["x_in"] = np.ascontiguousarray(
            x[c * WPC:(c + 1) * WPC].reshape(T, DIM))
        in_maps.append(m)
    return in_maps


_NC = None


def kernel(**inputs):
    global _NC
    if _NC is None:
        _NC = build_nc()
    in_maps = _prep_host(inputs)
    res = run_bass_kernel_spmd(
        _NC, in_maps, core_ids=list(range(N_CORES)),
        trace=bool(int(os.environ.get("KERNEL_TRACE", "0"))),
    )
    outs = [r["out"] for r in res.results]
    full = np.stack(outs, 0).reshape(BW, N, DIM).astype(np.float32)
    kernel.last_results = res
    return full


if __name__ == "__main__":
    import time
    t0 = time.time()
    nc = build_nc()
    print("built OK in %.1fs" % (time.time() - t0))
